# revision 31
# baseline (speedup 1.0000x reference)
"""KiloNeRF Trainium2 kernel: 4096 tiny MLPs, 512 points each, 8 NeuronCores.

Sharding: expert-parallel along the network axis (512 nets/core). Host-side
numpy packs per-core inputs into feature-major, PE-friendly layouts; the
device kernel is a stream of full-array block-diagonal matmuls (4 nets per
128-partition tile), bf16 inputs with f32 PSUM accumulation.

Structure (v3):
- The feature layer is folded into the direction layer on the host
  (feature is only an intermediate): W_eff = Wd_f @ Wf, b_eff = bd +
  Wd_f @ bf. 7 matmuls / 3584 streamed columns per 4-net group.
- All weights live in SBUF, preloaded once in ~11 big DMAs; per group the
  block-diagonal weight canvas is built with 6 small on-chip copies
  (vector/gpsimd), so the per-group DMA issue cost is 2 descriptors
  (one merged x load, one merged output store) instead of 13.
- Emission is software-pipelined with a one-group lag per layer so the
  PE queue never waits on the inter-layer relu (scalar/vector/gpsimd).
"""

import sys

sys.path.insert(0, "/opt/trn_rl_repo")

import numpy as np
import ml_dtypes

N_NET = 4096
P = 512
PC = 63
DC = 27
H = 32
NCORES = 8
NPC = N_NET // NCORES  # nets per core = 512
NPG = 4  # nets per group (one 128-partition tile)
G = NPC // NPG  # groups per core = 128
NCV = 5  # weight-canvas ring depth

BF16 = ml_dtypes.bfloat16

_nc_cache = {}


def _build_nc():
    import concourse.mybir as mybir
    import concourse.tile as tile
    from concourse import bacc

    nc = bacc.Bacc("TRN2")
    dt = mybir.dt
    AF = mybir.ActivationFunctionType
    ALU = mybir.AluOpType

    with tile.TileContext(nc) as tc:
        xall_d = nc.dram_tensor("xall", [128, G, 3 * P], dt.bfloat16, kind="ExternalInput")
        wd_d = nc.dram_tensor("wd", [4, 32, G, 96], dt.bfloat16, kind="ExternalInput")
        w0f_d = nc.dram_tensor("w0f", [128, G, 256], dt.bfloat16, kind="ExternalInput")
        wrwa_d = nc.dram_tensor("wrwa", [128, G, 32], dt.bfloat16, kind="ExternalInput")
        bia_d = nc.dram_tensor("bia", [128, G, 4], dt.float32, kind="ExternalInput")
        bout_d = nc.dram_tensor("bout", [16, G], dt.float32, kind="ExternalInput")
        out_d = nc.dram_tensor("out", [G, 16, P], dt.bfloat16, kind="ExternalOutput")

        with (
            tc.tile_pool(name="const", bufs=1) as constp,
            tc.tile_pool(name="cv", bufs=1) as cvp,
            tc.tile_pool(name="io", bufs=5) as io,
            tc.tile_pool(name="act", bufs=4) as actp,
            tc.tile_pool(name="ps0", bufs=2, space="PSUM") as ps0,
            tc.tile_pool(name="ps1", bufs=2, space="PSUM") as ps1,
            tc.tile_pool(name="psd", bufs=2, space="PSUM") as psd,
            tc.tile_pool(name="pso", bufs=2, space="PSUM") as pso,
        ):
            # ---- persistent weight store (SBUF-resident) ----
            WD = constp.tile([128, G * 96], dt.bfloat16, tag="WD")
            W0F = constp.tile([128, G * 256], dt.bfloat16, tag="W0F")
            WRWA = constp.tile([128, G * 32], dt.bfloat16, tag="WRWA")
            BIA = constp.tile([128, G * 4], dt.float32, tag="BIA")
            BOUT = constp.tile([16, G], dt.float32, tag="BOUT")
            # Preload on the scalar HWDGE queue (keeps the sync queue free
            # for the per-group x stream), in 8 interleaved group-range
            # chunks so group 0's weights land within a few us.
            nc.scalar.dma_start(out=BIA[:], in_=bia_d[:])
            nc.scalar.dma_start(out=BOUT[:], in_=bout_d[:])
            NCHUNK = 8
            ck = G // NCHUNK
            for g0 in range(0, G, ck):
                for j in range(4):
                    nc.scalar.dma_start(
                        out=WD[32 * j : 32 * j + 32, g0 * 96 : (g0 + ck) * 96],
                        in_=wd_d[j, :, g0 : g0 + ck],
                    )
                nc.scalar.dma_start(
                    out=W0F[:, g0 * 256 : (g0 + ck) * 256], in_=w0f_d[:, g0 : g0 + ck]
                )
                nc.scalar.dma_start(
                    out=WRWA[:, g0 * 32 : (g0 + ck) * 32], in_=wrwa_d[:, g0 : g0 + ck]
                )

            # ---- weight canvas supertiles, 8 groups per tile ----
            # Each group slot is 384 cols: [W1 | W_eff | Wd_d], block-diag
            # within each 128-col canvas. Scattered from WD by SBUF->SBUF
            # DMAs issued on the gpsimd SWDGE queue (issue cost only; the
            # transfer itself runs on the DMA engines).
            B = 8
            SLOT = 384
            NSUP = 4
            supers = []
            for i in range(NSUP):
                cv = cvp.tile([128, B * SLOT], dt.bfloat16, tag=f"cv{i}")
                nc.vector.memset(cv[:], 0.0)
                supers.append(cv)

            st = {}  # per-group live tiles

            def emit_xdma(g):
                xt = io.tile([128, 3 * P], dt.bfloat16, tag="xt")
                nc.sync.dma_start(out=xt[:], in_=xall_d[:, g])
                st.setdefault(g, {})["xt"] = xt

            def emit_scatter_batch(b):
                # one [32, B, 3, 32] SBUF->SBUF DMA per strip for groups
                # [Bb, Bb+B) into supertile b % NSUP
                cv = supers[b % NSUP]
                g0 = B * b
                for q in range(B):
                    if g0 + q < G:
                        st.setdefault(g0 + q, {})
                        st[g0 + q]["cv"] = cv
                        st[g0 + q]["cvq"] = SLOT * q
                for j in range(4):
                    nc.gpsimd.dma_start(
                        out=cv[32 * j : 32 * j + 32].rearrange(
                            "p (q m c) -> p q m c", q=B, m=3
                        )[:, :, :, 32 * j : 32 * j + 32],
                        in_=WD[32 * j : 32 * j + 32, 96 * g0 : 96 * (g0 + B)].rearrange(
                            "p (q m c) -> p q m c", q=B, m=3
                        ),
                    )

            def slot_mat(cv, q, m):
                # contiguous 128-col block-diag canvas m (0=W1, 1=W_eff, 2=Wd_d)
                return cv[:, q + 128 * m : q + 128 * m + 128]

            def emit_l0(g):
                s = st[g]
                xt = s["xt"]
                p_l0 = ps0.tile([128, P], dt.float32, tag="l0")
                # true accumulation pair over the full 128 partitions
                # (zero-padded stationaries) so the PE fuses the dispatch
                nc.tensor.matmul(p_l0[:], lhsT=W0F[:, 256 * g : 256 * g + 128], rhs=xt[:, 0:P], start=True, stop=False)
                nc.tensor.matmul(p_l0[:], lhsT=W0F[:, 256 * g + 128 : 256 * g + 256], rhs=xt[:, P : 2 * P], start=False, stop=True)
                h1 = actp.tile([128, P], dt.bfloat16, tag="h1")
                nc.scalar.activation(h1[:], p_l0[:], AF.Relu, bias=BIA[:, 4 * g : 4 * g + 1], scale=1.0)
                s["h1"] = h1

            def emit_l1(g):
                s = st[g]
                cv, q = s["cv"], s["cvq"]
                p_l1 = ps1.tile([128, P], dt.float32, tag="l1")
                nc.tensor.matmul(p_l1[:], lhsT=slot_mat(cv, q, 0), rhs=s["h1"][:], start=True, stop=True)
                h2 = actp.tile([128, P], dt.bfloat16, tag="h2")
                nc.vector.tensor_scalar(h2[:], p_l1[:], BIA[:, 4 * g + 1 : 4 * g + 2], 0.0, op0=ALU.add, op1=ALU.max)
                s["h2"] = h2

            def emit_ld(g):
                s = st[g]
                cv, q = s["cv"], s["cvq"]
                p_ld = psd.tile([128, P], dt.float32, tag="ld")
                nc.tensor.matmul(p_ld[:], lhsT=slot_mat(cv, q, 1), rhs=s["h2"][:], start=True, stop=False)
                nc.tensor.matmul(p_ld[:], lhsT=slot_mat(cv, q, 2), rhs=s["xt"][:, 2 * P : 3 * P], start=False, stop=True)
                h3 = actp.tile([128, P], dt.bfloat16, tag="h3")
                nc.scalar.activation(h3[:], p_ld[:], AF.Relu, bias=BIA[:, 4 * g + 2 : 4 * g + 3], scale=1.0)
                s["h3"] = h3

            def emit_lout(g):
                s = st[g]
                p_lo = pso.tile([16, P], dt.float32, tag="lo")
                nc.tensor.matmul(p_lo[:], lhsT=WRWA[:, 32 * g : 32 * g + 16], rhs=s["h3"][:], start=True, stop=False)
                nc.tensor.matmul(p_lo[:], lhsT=WRWA[:, 32 * g + 16 : 32 * g + 32], rhs=s["h2"][:], start=False, stop=True)
                ob = actp.tile([16, P], dt.bfloat16, tag="ob")
                nc.vector.tensor_scalar_add(ob[:], p_lo[:], BOUT[:, g : g + 1])
                nc.sync.dma_start(out=out_d[g], in_=ob[:])
                del st[g]

            # ---- software-pipelined emission, one-group lag per layer ----
            emit_scatter_batch(0)
            emit_xdma(0)
            for t in range(G + 3):
                if t + 1 < G:
                    emit_xdma(t + 1)
                if t < G:
                    emit_l0(t)
                if t - 1 >= 0 and t - 1 < G:
                    emit_l1(t - 1)
                if t - 2 >= 0 and t - 2 < G:
                    emit_ld(t - 2)
                if t - 3 >= 0 and t - 3 < G:
                    emit_lout(t - 3)
                if (t + 6) % B == 0 and t + 6 < G:
                    emit_scatter_batch((t + 6) // B)

    nc.compile()
    return nc


def _pack_core(c, x, W0, b0, W1, b1, Wa, ba, Wf, bf, Wd, bd, Wr, br):
    lo, hi = c * NPC, (c + 1) * NPC
    xT = np.ascontiguousarray(
        x[lo:hi].transpose(0, 2, 1)
    )  # [512, 90, 512] f32 feature-major

    xarr = np.zeros((128, G, 3, P), dtype=BF16)
    pt = xT[:, :PC, :].astype(BF16).reshape(G, 4, PC, P)
    xa = xarr.transpose(1, 2, 0, 3)  # [G, 3, 128, P] view
    xa[:, 0, 0:PC] = pt[:, 0]
    xa[:, 0, 64 : 64 + PC] = pt[:, 1]
    xa[:, 1, 0:PC] = pt[:, 2]
    xa[:, 1, 64 : 64 + PC] = pt[:, 3]
    dd = xT[:, PC:, :].astype(BF16).reshape(G, 4, DC, P)
    for j in range(4):
        xa[:, 2, 32 * j : 32 * j + DC] = dd[:, j]
    xall = np.ascontiguousarray(xarr.reshape(128, G, 3 * P))

    # w0 stationaries as a zero-padded accumulation pair: [128, G, 2, 128]
    # pair 0 (rhs=pos0) fills out cols 0:64 (nets 0,1), pair 1 (rhs=pos1)
    # fills out cols 64:128 (nets 2,3); the other half is zero.
    w0T = W0[lo:hi].transpose(0, 2, 1).astype(BF16).reshape(G, 4, PC, H)
    w0f = np.zeros((G, 2, 128, 128), dtype=BF16)
    for j in range(4):
        r = 64 * (j % 2)
        w0f[:, j // 2, r : r + PC, 32 * j : 32 * j + 32] = w0T[:, j]
    w0f = np.ascontiguousarray(w0f.transpose(2, 0, 1, 3).reshape(128, G, 256))

    # Fold the feature layer into the direction layer:
    #   h3pre = (Wd_f Wf) h2 + Wd_d dir + (bd + Wd_f bf)
    Wd_f = Wd[lo:hi, :, :H]
    Wd_d = Wd[lo:hi, :, H:]
    w_eff = np.matmul(Wd_f, Wf[lo:hi])
    b_eff = bd[lo:hi] + np.einsum("noi,ni->no", Wd_f, bf[lo:hi])

    # wd compact: [G, 4, 32, 3, 32] -> [4, 32, G, 96]  (W1 | W_eff | Wd_d)
    wdiag = np.zeros((G, 4, 32, 3, 32), dtype=BF16)
    w1T = W1[lo:hi].transpose(0, 2, 1).astype(BF16).reshape(G, 4, H, H)
    weffT = w_eff.transpose(0, 2, 1).astype(BF16).reshape(G, 4, H, H)
    wddT = Wd_d.transpose(0, 2, 1).astype(BF16).reshape(G, 4, DC, H)
    wdiag[:, :, :, 0, :] = w1T
    wdiag[:, :, :, 1, :] = weffT
    wdiag[:, :, :DC, 2, :] = wddT
    wd = np.ascontiguousarray(wdiag.transpose(1, 2, 0, 3, 4).reshape(4, 32, G, 96))

    # Wr/Wa full stationaries [128, 16] each: col 4j+c = output c of net j,
    # reading rows 32j:32j+32.
    wrT = Wr[lo:hi].transpose(0, 2, 1).astype(BF16).reshape(G, 4, H, 3)
    waT = Wa[lo:hi].transpose(0, 2, 1).astype(BF16).reshape(G, 4, H, 1)
    wrwa = np.zeros((G, 128, 32), dtype=BF16)
    for j in range(4):
        wrwa[:, 32 * j : 32 * j + 32, 4 * j : 4 * j + 3] = wrT[:, j]
        wrwa[:, 32 * j : 32 * j + 32, 16 + 4 * j + 3 : 16 + 4 * j + 4] = waT[:, j]
    wrwa = np.ascontiguousarray(wrwa.transpose(1, 0, 2))

    bias = np.zeros((G, 128, 4), dtype=np.float32)
    bias[:, :, 0] = b0[lo:hi].reshape(G, 128)
    bias[:, :, 1] = b1[lo:hi].reshape(G, 128)
    bias[:, :, 2] = b_eff.reshape(G, 128)
    bia = np.ascontiguousarray(bias.transpose(1, 0, 2))

    bout = np.zeros((G, 4, 4), dtype=np.float32)
    bout[:, :, 0:3] = br[lo:hi].reshape(G, 4, 3)
    bout[:, :, 3] = ba[lo:hi].reshape(G, 4)
    bout = np.ascontiguousarray(bout.reshape(G, 16).T)

    return {
        "xall": xall,
        "wd": wd,
        "w0f": w0f,
        "wrwa": wrwa,
        "bia": bia,
        "bout": bout,
    }


def kernel(**inputs):
    from concourse.bass_utils import run_bass_kernel_spmd

    if "nc" not in _nc_cache:
        _nc_cache["nc"] = _build_nc()
    nc = _nc_cache["nc"]

    from concurrent.futures import ThreadPoolExecutor

    with ThreadPoolExecutor(max_workers=8) as ex:
        in_maps = list(ex.map(lambda c: _pack_core(c, **inputs), range(NCORES)))

    res = run_bass_kernel_spmd(nc, in_maps, core_ids=list(range(NCORES)))

    out = np.empty((N_NET, P, 4), dtype=np.float32)
    for c in range(NCORES):
        o = res.results[c]["out"].astype(np.float32)  # [G, 16, P] bf16 -> f32
        out[c * NPC : (c + 1) * NPC] = o.reshape(G * NPG, 4, P).transpose(0, 2, 1)
    return out


# revision 34
# speedup vs baseline: 1.0858x; 1.0858x over previous
"""KiloNeRF Trainium2 kernel: 4096 tiny MLPs, 512 points each, 8 NeuronCores.

Sharding: expert-parallel along the network axis (512 nets/core). Host-side
numpy packs per-core inputs into feature-major, PE-friendly layouts; the
device kernel is a stream of full-array block-diagonal matmuls (4 nets per
128-partition tile), bf16 inputs with f32 PSUM accumulation.

Structure (v3):
- The feature layer is folded into the direction layer on the host
  (feature is only an intermediate): W_eff = Wd_f @ Wf, b_eff = bd +
  Wd_f @ bf. 7 matmuls / 3584 streamed columns per 4-net group.
- All weights live in SBUF, preloaded once in ~11 big DMAs; per group the
  block-diagonal weight canvas is built with 6 small on-chip copies
  (vector/gpsimd), so the per-group DMA issue cost is 2 descriptors
  (one merged x load, one merged output store) instead of 13.
- Emission is software-pipelined with a one-group lag per layer so the
  PE queue never waits on the inter-layer relu (scalar/vector/gpsimd).
"""

import sys

sys.path.insert(0, "/opt/trn_rl_repo")

import numpy as np
import ml_dtypes

N_NET = 4096
P = 512
PC = 63
DC = 27
H = 32
NCORES = 8
NPC = N_NET // NCORES  # nets per core = 512
NPG = 4  # nets per group (one 128-partition tile)
G = NPC // NPG  # groups per core = 128
NCV = 5  # weight-canvas ring depth

BF16 = ml_dtypes.bfloat16

_nc_cache = {}


def _build_nc():
    import concourse.mybir as mybir
    import concourse.tile as tile
    from concourse import bacc

    nc = bacc.Bacc("TRN2")
    dt = mybir.dt
    AF = mybir.ActivationFunctionType
    ALU = mybir.AluOpType

    with tile.TileContext(nc) as tc:
        xall_d = nc.dram_tensor("xall", [128, G, 3 * P], dt.bfloat16, kind="ExternalInput")
        wd_d = nc.dram_tensor("wd", [4, 32, G, 96], dt.bfloat16, kind="ExternalInput")
        w0f_d = nc.dram_tensor("w0f", [128, G, 256], dt.bfloat16, kind="ExternalInput")
        wrwa_d = nc.dram_tensor("wrwa", [128, G, 32], dt.bfloat16, kind="ExternalInput")
        bia_d = nc.dram_tensor("bia", [128, G, 4], dt.float32, kind="ExternalInput")
        bout_d = nc.dram_tensor("bout", [16, G], dt.float32, kind="ExternalInput")
        out_d = nc.dram_tensor("out", [G, 16, P], dt.bfloat16, kind="ExternalOutput")

        with (
            tc.tile_pool(name="const", bufs=1) as constp,
            tc.tile_pool(name="cv", bufs=1) as cvp,
            tc.tile_pool(name="io", bufs=6) as io,
            tc.tile_pool(name="act", bufs=5) as actp,
            tc.tile_pool(name="ps0", bufs=2, space="PSUM") as ps0,
            tc.tile_pool(name="ps1", bufs=2, space="PSUM") as ps1,
            tc.tile_pool(name="psd", bufs=2, space="PSUM") as psd,
            tc.tile_pool(name="pso", bufs=2, space="PSUM") as pso,
        ):
            # ---- persistent weight store (SBUF-resident) ----
            WD = constp.tile([128, G * 96], dt.bfloat16, tag="WD")
            W0F = constp.tile([128, G * 256], dt.bfloat16, tag="W0F")
            WRWA = constp.tile([128, G * 32], dt.bfloat16, tag="WRWA")
            BIA = constp.tile([128, G * 4], dt.float32, tag="BIA")
            BOUT = constp.tile([16, G], dt.float32, tag="BOUT")
            # Weight preload happens in 16-group chunks; chunk 0 is pushed
            # in the prologue, later chunks are interleaved into the main
            # loop ~16 steps ahead of first use so no queue sees a burst.
            nc.scalar.dma_start(out=BIA[:], in_=bia_d[:])
            nc.scalar.dma_start(out=BOUT[:], in_=bout_d[:])
            CK = 16

            def emit_preload_chunk(g0):
                for j in range(4):
                    nc.gpsimd.dma_start(
                        out=WD[32 * j : 32 * j + 32, g0 * 96 : (g0 + CK) * 96],
                        in_=wd_d[j, :, g0 : g0 + CK],
                    )
                nc.scalar.dma_start(
                    out=W0F[:, g0 * 256 : (g0 + CK) * 256], in_=w0f_d[:, g0 : g0 + CK]
                )
                nc.scalar.dma_start(
                    out=WRWA[:, g0 * 32 : (g0 + CK) * 32], in_=wrwa_d[:, g0 : g0 + CK]
                )

            emit_preload_chunk(0)

            # ---- weight canvas supertiles, 8 groups per tile ----
            # Each group slot is 384 cols: [W1 | W_eff | Wd_d], block-diag
            # within each 128-col canvas. Scattered from WD by SBUF->SBUF
            # DMAs issued on the gpsimd SWDGE queue (issue cost only; the
            # transfer itself runs on the DMA engines).
            B = 8
            SLOT = 384
            NSUP = 4
            supers = []
            for i in range(NSUP):
                cv = cvp.tile([128, B * SLOT], dt.bfloat16, tag=f"cv{i}")
                nc.vector.memset(cv[:], 0.0)
                supers.append(cv)

            st = {}  # per-group live tiles

            def emit_xdma(g):
                xt = io.tile([128, 3 * P], dt.bfloat16, tag="xt")
                nc.sync.dma_start(out=xt[:], in_=xall_d[:, g])
                st.setdefault(g, {})["xt"] = xt

            def emit_scatter_batch(b):
                # one [32, B, 3, 32] SBUF->SBUF DMA per strip for groups
                # [Bb, Bb+B) into supertile b % NSUP
                cv = supers[b % NSUP]
                g0 = B * b
                for q in range(B):
                    if g0 + q < G:
                        st.setdefault(g0 + q, {})
                        st[g0 + q]["cv"] = cv
                        st[g0 + q]["cvq"] = SLOT * q
                for j in range(4):
                    nc.gpsimd.dma_start(
                        out=cv[32 * j : 32 * j + 32].rearrange(
                            "p (q m c) -> p q m c", q=B, m=3
                        )[:, :, :, 32 * j : 32 * j + 32],
                        in_=WD[32 * j : 32 * j + 32, 96 * g0 : 96 * (g0 + B)].rearrange(
                            "p (q m c) -> p q m c", q=B, m=3
                        ),
                    )

            def slot_mat(cv, q, m):
                # contiguous 128-col block-diag canvas m (0=W1, 1=W_eff, 2=Wd_d)
                return cv[:, q + 128 * m : q + 128 * m + 128]

            def emit_l0(g):
                s = st[g]
                xt = s["xt"]
                p_l0 = ps0.tile([128, P], dt.float32, tag="l0")
                # true accumulation pair over the full 128 partitions
                # (zero-padded stationaries) so the PE fuses the dispatch
                nc.tensor.matmul(p_l0[:], lhsT=W0F[:, 256 * g : 256 * g + 128], rhs=xt[:, 0:P], start=True, stop=False)
                nc.tensor.matmul(p_l0[:], lhsT=W0F[:, 256 * g + 128 : 256 * g + 256], rhs=xt[:, P : 2 * P], start=False, stop=True)
                h1 = actp.tile([128, P], dt.bfloat16, tag="h1")
                nc.scalar.activation(h1[:], p_l0[:], AF.Relu, bias=BIA[:, 4 * g : 4 * g + 1], scale=1.0)
                s["h1"] = h1

            def emit_l1(g):
                s = st[g]
                cv, q = s["cv"], s["cvq"]
                p_l1 = ps1.tile([128, P], dt.float32, tag="l1")
                nc.tensor.matmul(p_l1[:], lhsT=slot_mat(cv, q, 0), rhs=s["h1"][:], start=True, stop=True)
                h2 = actp.tile([128, P], dt.bfloat16, tag="h2")
                nc.vector.tensor_scalar(h2[:], p_l1[:], BIA[:, 4 * g + 1 : 4 * g + 2], 0.0, op0=ALU.add, op1=ALU.max)
                s["h2"] = h2

            def emit_ld(g):
                s = st[g]
                cv, q = s["cv"], s["cvq"]
                p_ld = psd.tile([128, P], dt.float32, tag="ld")
                nc.tensor.matmul(p_ld[:], lhsT=slot_mat(cv, q, 1), rhs=s["h2"][:], start=True, stop=False)
                nc.tensor.matmul(p_ld[:], lhsT=slot_mat(cv, q, 2), rhs=s["xt"][:, 2 * P : 3 * P], start=False, stop=True)
                h3 = actp.tile([128, P], dt.bfloat16, tag="h3")
                nc.scalar.activation(h3[:], p_ld[:], AF.Relu, bias=BIA[:, 4 * g + 2 : 4 * g + 3], scale=1.0)
                s["h3"] = h3

            def emit_lout(g):
                s = st[g]
                p_lo = pso.tile([16, P], dt.float32, tag="lo")
                nc.tensor.matmul(p_lo[:], lhsT=WRWA[:, 32 * g : 32 * g + 16], rhs=s["h3"][:], start=True, stop=False)
                nc.tensor.matmul(p_lo[:], lhsT=WRWA[:, 32 * g + 16 : 32 * g + 32], rhs=s["h2"][:], start=False, stop=True)
                ob = actp.tile([16, P], dt.bfloat16, tag="ob")
                nc.vector.tensor_scalar_add(ob[:], p_lo[:], BOUT[:, g : g + 1])
                nc.sync.dma_start(out=out_d[g], in_=ob[:])
                del st[g]

            # ---- software-pipelined emission, one-group lag per layer ----
            emit_scatter_batch(0)
            emit_scatter_batch(1)
            emit_xdma(0)
            for t in range(G + 3):
                if t + 1 < G:
                    emit_xdma(t + 1)
                if t < G:
                    emit_l0(t)
                if t - 1 >= 0 and t - 1 < G:
                    emit_l1(t - 1)
                if t - 2 >= 0 and t - 2 < G:
                    emit_ld(t - 2)
                if t - 3 >= 0 and t - 3 < G:
                    emit_lout(t - 3)
                if t % CK == 0 and t + CK < G:
                    emit_preload_chunk(t + CK)
                if (t + 14) % B == 0 and t + 14 < G:
                    emit_scatter_batch((t + 14) // B)

    nc.compile()
    return nc


def _pack_core(c, x, W0, b0, W1, b1, Wa, ba, Wf, bf, Wd, bd, Wr, br):
    lo, hi = c * NPC, (c + 1) * NPC
    xT = np.ascontiguousarray(
        x[lo:hi].transpose(0, 2, 1)
    )  # [512, 90, 512] f32 feature-major

    xarr = np.zeros((128, G, 3, P), dtype=BF16)
    pt = xT[:, :PC, :].astype(BF16).reshape(G, 4, PC, P)
    xa = xarr.transpose(1, 2, 0, 3)  # [G, 3, 128, P] view
    xa[:, 0, 0:PC] = pt[:, 0]
    xa[:, 0, 64 : 64 + PC] = pt[:, 1]
    xa[:, 1, 0:PC] = pt[:, 2]
    xa[:, 1, 64 : 64 + PC] = pt[:, 3]
    dd = xT[:, PC:, :].astype(BF16).reshape(G, 4, DC, P)
    for j in range(4):
        xa[:, 2, 32 * j : 32 * j + DC] = dd[:, j]
    xall = np.ascontiguousarray(xarr.reshape(128, G, 3 * P))

    # w0 stationaries as a zero-padded accumulation pair: [128, G, 2, 128]
    # pair 0 (rhs=pos0) fills out cols 0:64 (nets 0,1), pair 1 (rhs=pos1)
    # fills out cols 64:128 (nets 2,3); the other half is zero.
    w0T = W0[lo:hi].transpose(0, 2, 1).astype(BF16).reshape(G, 4, PC, H)
    w0f = np.zeros((G, 2, 128, 128), dtype=BF16)
    for j in range(4):
        r = 64 * (j % 2)
        w0f[:, j // 2, r : r + PC, 32 * j : 32 * j + 32] = w0T[:, j]
    w0f = np.ascontiguousarray(w0f.transpose(2, 0, 1, 3).reshape(128, G, 256))

    # Fold the feature layer into the direction layer:
    #   h3pre = (Wd_f Wf) h2 + Wd_d dir + (bd + Wd_f bf)
    Wd_f = Wd[lo:hi, :, :H]
    Wd_d = Wd[lo:hi, :, H:]
    w_eff = np.matmul(Wd_f, Wf[lo:hi])
    b_eff = bd[lo:hi] + np.einsum("noi,ni->no", Wd_f, bf[lo:hi])

    # wd compact: [G, 4, 32, 3, 32] -> [4, 32, G, 96]  (W1 | W_eff | Wd_d)
    wdiag = np.zeros((G, 4, 32, 3, 32), dtype=BF16)
    w1T = W1[lo:hi].transpose(0, 2, 1).astype(BF16).reshape(G, 4, H, H)
    weffT = w_eff.transpose(0, 2, 1).astype(BF16).reshape(G, 4, H, H)
    wddT = Wd_d.transpose(0, 2, 1).astype(BF16).reshape(G, 4, DC, H)
    wdiag[:, :, :, 0, :] = w1T
    wdiag[:, :, :, 1, :] = weffT
    wdiag[:, :, :DC, 2, :] = wddT
    wd = np.ascontiguousarray(wdiag.transpose(1, 2, 0, 3, 4).reshape(4, 32, G, 96))

    # Wr/Wa full stationaries [128, 16] each: col 4j+c = output c of net j,
    # reading rows 32j:32j+32.
    wrT = Wr[lo:hi].transpose(0, 2, 1).astype(BF16).reshape(G, 4, H, 3)
    waT = Wa[lo:hi].transpose(0, 2, 1).astype(BF16).reshape(G, 4, H, 1)
    wrwa = np.zeros((G, 128, 32), dtype=BF16)
    for j in range(4):
        wrwa[:, 32 * j : 32 * j + 32, 4 * j : 4 * j + 3] = wrT[:, j]
        wrwa[:, 32 * j : 32 * j + 32, 16 + 4 * j + 3 : 16 + 4 * j + 4] = waT[:, j]
    wrwa = np.ascontiguousarray(wrwa.transpose(1, 0, 2))

    bias = np.zeros((G, 128, 4), dtype=np.float32)
    bias[:, :, 0] = b0[lo:hi].reshape(G, 128)
    bias[:, :, 1] = b1[lo:hi].reshape(G, 128)
    bias[:, :, 2] = b_eff.reshape(G, 128)
    bia = np.ascontiguousarray(bias.transpose(1, 0, 2))

    bout = np.zeros((G, 4, 4), dtype=np.float32)
    bout[:, :, 0:3] = br[lo:hi].reshape(G, 4, 3)
    bout[:, :, 3] = ba[lo:hi].reshape(G, 4)
    bout = np.ascontiguousarray(bout.reshape(G, 16).T)

    return {
        "xall": xall,
        "wd": wd,
        "w0f": w0f,
        "wrwa": wrwa,
        "bia": bia,
        "bout": bout,
    }


def kernel(**inputs):
    from concourse.bass_utils import run_bass_kernel_spmd

    if "nc" not in _nc_cache:
        _nc_cache["nc"] = _build_nc()
    nc = _nc_cache["nc"]

    from concurrent.futures import ThreadPoolExecutor

    with ThreadPoolExecutor(max_workers=8) as ex:
        in_maps = list(ex.map(lambda c: _pack_core(c, **inputs), range(NCORES)))

    res = run_bass_kernel_spmd(nc, in_maps, core_ids=list(range(NCORES)))

    out = np.empty((N_NET, P, 4), dtype=np.float32)
    for c in range(NCORES):
        o = res.results[c]["out"].astype(np.float32)  # [G, 16, P] bf16 -> f32
        out[c * NPC : (c + 1) * NPC] = o.reshape(G * NPG, 4, P).transpose(0, 2, 1)
    return out
